# revision 34
# baseline (speedup 1.0000x reference)
"""HGNN layer (hypergraph message passing) Trainium2 kernel, 8 NeuronCores.

Sharding: one graph per PAIR of cores; core owns one e-half AND one n-half.
The incidence matrix ships as fp8e4m3 (0/1 exact) and feeds the PE directly
as the moving operand against bf16 stationaries -- no on-chip casts.
Hyperedges are permuted unmasked-first per half on the host so the
attention-masked H@u pass contracts only ~half the e tiles.

Stage boundaries are pair exchanges: the first (X1) is a 1MB AllReduce of
the full-width a1 partial (local-first layout + swap-halves on receive keeps
it SPMD-uniform); X2..X4 are 0.5MB AllReduce+subtract swaps whose remote
halves overlap the next pass's local-half matmuls. The output needs no
collective: each core emits its own n-half and the host concatenates.

DMA: big streams use 2MB chunks alternated across the sync and scalar
queues (per-DMA fixed overhead ~1.8us would cap a single queue at
~210GB/s); collective staging lives on gpsimd; htr1 on vector.
"""

import numpy as np

B, N, E, D = 4, 4096, 4096, 128
HALF = N // 2
NCORES = 8
PAIRS = [[0, 1], [2, 3], [4, 5], [6, 7]]
BN_EPS = 1e-5
F = 512                 # psum block free size
NT = N // 128           # 32 k-tiles over a full 4096 contraction
HT = HALF // 128        # 16 tiles over a half
CH = 8                  # tiles per 2MB stream chunk (fp8: 8x[128,2048])

_CACHE = {}


def _build(T1):
    import concourse.bacc as bacc
    import concourse.mybir as mybir
    import concourse.tile as tile
    from concourse.masks import make_identity
    from contextlib import ExitStack

    fp32 = mybir.dt.float32
    bf16 = mybir.dt.bfloat16
    f8e4 = mybir.dt.float8e4
    Act = mybir.ActivationFunctionType
    Alu = mybir.AluOpType

    T2 = HT - T1            # masked-complement e tiles per half
    NB = HALF // F          # 4 psum blocks per half-width output
    EB = min(NB, (T1 * 128 + F - 1) // F)   # s2a blocks (cover u cols)
    # a5 tail stream: own-masked T2 + remote packed T1 + remote masked T2
    A5T = 2 * T2 + T1
    CH5 = 6                 # htr5 tiles per chunk (1.5MB, recycles hcA bufs)
    PAD5 = ((A5T + CH5 - 1) // CH5) * CH5
    H1C = (T1 + 2) // 3     # htr1 chunks (3 tiles of [128, N] each)

    nc = bacc.Bacc("TRN2", target_bir_lowering=False, debug=False,
                   num_devices=NCORES)

    xt_d = nc.dram_tensor("xt", [128, N], bf16, kind="ExternalInput")
    pk_d = nc.dram_tensor("pk", [128, 5 + T1], fp32, kind="ExternalInput")
    hcolA_d = nc.dram_tensor("hcolA", [128, NT * 3 * F], f8e4,
                             kind="ExternalInput")
    hcolB_d = nc.dram_tensor("hcolB", [128, NT * F], f8e4,
                             kind="ExternalInput")
    htr1_d = nc.dram_tensor("htr1", [128, 3 * H1C * N], f8e4,
                            kind="ExternalInput")
    htr5_d = nc.dram_tensor("htr5", [128, max(1, PAD5) * HALF], f8e4,
                            kind="ExternalInput")
    dvt_d = nc.dram_tensor("dvt", [128, NT * HALF], bf16, kind="ExternalInput")
    det_d = nc.dram_tensor("det", [128, NT * HALF], bf16, kind="ExternalInput")
    w_d = nc.dram_tensor("w", [D, D], bf16, kind="ExternalInput")
    y_d = nc.dram_tensor("y", [D, HALF], bf16, kind="ExternalOutput")

    with tile.TileContext(nc) as tc, ExitStack() as ctx:
        const = ctx.enter_context(tc.tile_pool(name="const", bufs=1))
        hcA = ctx.enter_context(tc.tile_pool(name="hcA", bufs=4))
        hcB = ctx.enter_context(tc.tile_pool(name="hcB", bufs=1))
        h1p = ctx.enter_context(tc.tile_pool(name="h1p", bufs=max(1, H1C)))
        # (h1p bufs = H1C chunks of [128, 4N])
        st = ctx.enter_context(tc.tile_pool(name="st", bufs=4))
        nt1 = ctx.enter_context(tc.tile_pool(name="nt1", bufs=1))
        nt2 = ctx.enter_context(tc.tile_pool(name="nt2", bufs=1))
        tset = ctx.enter_context(tc.tile_pool(name="tset", bufs=2))
        xset = ctx.enter_context(tc.tile_pool(name="xset", bufs=2))
        med = ctx.enter_context(tc.tile_pool(name="med", bufs=1))
        acc = ctx.enter_context(tc.tile_pool(name="acc", bufs=1, space="PSUM"))
        pst = ctx.enter_context(tc.tile_pool(name="pst", bufs=4, space="PSUM"))
        dram = ctx.enter_context(tc.tile_pool(name="dram", bufs=1, space="DRAM"))

        # ---- startup DMAs. sync: xt + hcol evens; scalar: w/attn + odds.
        xt_t = nt1.tile([128, N], bf16, tag="nt1", name="xt_s")
        nc.sync.dma_start(out=xt_t[:, :1024], in_=xt_d.ap()[:, :1024])
        w_t = const.tile([D, D], bf16)
        nc.scalar.dma_start(out=w_t[:], in_=w_d.ap())
        pk_t = const.tile([128, 5 + T1], fp32)
        nc.scalar.dma_start(out=pk_t[:], in_=pk_d.ap())
        eps_t = pk_t[:, 0:1]
        bng_t = pk_t[:, 1:2]
        bnb_t = pk_t[:, 2:3]
        bnm_t = pk_t[:, 3:4]
        bnv_t = pk_t[:, 4:5]
        attn_t = pk_t[:, 5:]

        identb = const.tile([128, 128], bf16)
        make_identity(nc, identb)

        hcA_tiles, hcB_tiles = [], []
        for c in range(4):
            t = hcA.tile([128, CH * 3 * F], f8e4, tag="hcA", name=f"hcA{c}")
            nc.sync.dma_start(
                out=t[:],
                in_=hcolA_d.ap()[:, c * CH * 3 * F:(c + 1) * CH * 3 * F])
            hcA_tiles.append(t)
            if c == 1:
                nc.sync.dma_start(out=xt_t[:, 1024:],
                                  in_=xt_d.ap()[:, 1024:])

        def hcol_bap(j, b):
            if b < 3:
                base = (j % CH) * 3 * F + b * F
                return hcA_tiles[j // CH][:, base:base + F]
            return hcB_t[:, j * F:(j + 1) * F]

        # htr1: H^T[own packed e, n local-first] fp8, resident (a1 +
        # a5-own). On sync AFTER hcA: the DMA engines serve FIFO, so the
        # s2-critical hcA stream completes first.
        htr1_tiles = []
        for c in range(H1C):
            t = h1p.tile([128, 3 * N], f8e4, tag="h1p", name=f"h1{c}")
            nc.sync.dma_start(
                out=t[:], in_=htr1_d.ap()[:, c * 3 * N:(c + 1) * 3 * N])
            htr1_tiles.append(t)
        hcB_t = hcB.tile([128, NT * F], f8e4, tag="hcB", name="hcB_s")
        nc.sync.dma_start(out=hcB_t[:], in_=hcolB_d.ap())

        def htr1_ap(t_, c0, c1):
            return htr1_tiles[t_ // 3][:, (t_ % 3) * N + c0:(t_ % 3) * N + c1]

        def acc_t(b, nm):
            return acc.tile([128, F], fp32, tag=f"acc{b}", name=nm)

        def tT(nm):
            return tset.tile([D, HALF], bf16, tag="tset", name=nm)

        def transpose_to(dst_ap, src_ap, scale=None):
            p = pst.tile([128, 128], bf16, tag="pst")
            nc.tensor.transpose(p[:], src_ap, identb[:])
            if scale is None:
                nc.vector.tensor_copy(dst_ap, p[:])
            else:
                nc.vector.tensor_scalar_mul(dst_ap, p[:], scale)

        # X2..X4 exchange helpers: pair AllReduce + subtract trick
        def exchange_start(nm, own_ap, W):
            cin = dram.tile([D, W], bf16, tag=nm + "i")
            cout = dram.tile([D, W], bf16, tag=nm + "o")
            nc.gpsimd.dma_start(out=cin[:], in_=own_ap)
            nc.gpsimd.collective_compute(
                "AllReduce", Alu.add, replica_groups=PAIRS,
                ins=[cin.opt()], outs=[cout.opt()])
            s = xset.tile([D, W], bf16, tag="xset", name=nm + "s",
                          padded_shape=[D, HALF])
            r = xset.tile([D, W], bf16, tag="xset", name=nm + "r",
                          padded_shape=[D, HALF])
            return (cout, own_ap, s, r, W)

        def exchange_recv(h, p0, p1):
            cout, own_ap, s, r, W = h
            nc.gpsimd.dma_start(out=s[:, p0:p1], in_=cout[:][:, p0:p1])
            nc.vector.tensor_tensor(r[:, p0:p1], s[:, p0:p1],
                                    own_ap[:, p0:p1], op=Alu.subtract)
            return r

        def recv_transpose(h, dst, base):
            for p in range(2):
                p0, p1 = p * 8 * 128, (p + 1) * 8 * 128
                rem = exchange_recv(h, p0, p1)
                for j in range(p * 8, (p + 1) * 8):
                    transpose_to(dst[:, (base + j) * 128:(base + j + 1) * 128],
                                 rem[:, j * 128:(j + 1) * 128])

        # ------- s2a: m2/hxw blocks 0..EB-1 (covers packed-attn cols) -----
        m2T = tT("m2T")
        hxwT = med.tile([D, HALF], bf16, tag="hxwT")

        def s2_blocks(b0, b1, nm):
            ps = [acc_t(b, f"m2{nm}{b}") for b in range(b0, b1)]
            for j in range(NT):
                for i, b in enumerate(range(b0, b1)):
                    nc.tensor.matmul(ps[i][:],
                                     xt_t[:, j * 128:(j + 1) * 128],
                                     hcol_bap(j, b),
                                     start=(j == 0), stop=(j == NT - 1))
            for i, b in enumerate(range(b0, b1)):
                sl = slice(b * F, (b + 1) * F)
                nc.vector.tensor_copy(m2T[:, sl], ps[i][:])
            wps = [acc_t(b, f"hxw{nm}{b}") for b in range(b0, b1)]
            for i, b in enumerate(range(b0, b1)):
                sl = slice(b * F, (b + 1) * F)
                nc.tensor.matmul(wps[i][:], w_t[:], m2T[:, sl],
                                 start=True, stop=True)
                nc.vector.tensor_copy(hxwT[:, sl], wps[i][:])

        s2_blocks(0, EB, "a")

        # ------- u tiles [e, d] (own packed only) -------------------------
        u_t = med.tile([128, T1 * 128], bf16, tag="u_t")
        for t in range(T1):
            transpose_to(u_t[:, t * 128:(t + 1) * 128],
                         hxwT[:, t * 128:(t + 1) * 128],
                         scale=attn_t[:, t:t + 1])

        # ------- a1: partial (H[:, e_own] @ u_own).T over FULL n (LF) -----
        # Two sequential column-group passes (4 psum banks each); each
        # group's half is AllReduced as soon as its copies land (X1a/X1b).
        # sum - own = peer partial in peer-LF layout; combined half =
        # own[other half] + rem[this half] (LF swap), kept in a scratch
        # tile so h1aTf stays pristine for the second subtract.
        h1aTf = nt2.tile([128, N], bf16, tag="nt2", name="h1aTf")
        x1 = [None, None]

        def a1_group(g):
            ps = [acc_t(b, f"h1a{g}{b}") for b in range(NB)]
            for t in range(T1):
                for b in range(NB):
                    c0 = g * HALF + b * F
                    nc.tensor.matmul(ps[b][:], u_t[:, t * 128:(t + 1) * 128],
                                     htr1_ap(t, c0, c0 + F),
                                     start=(t == 0), stop=(t == T1 - 1))
            for b in range(NB):
                sl = slice(g * HALF + b * F, g * HALF + (b + 1) * F)
                nc.vector.tensor_copy(h1aTf[:, sl], ps[b][:])

        a1_group(0)
        x1[0] = exchange_start("x1a", h1aTf[:, :HALF], HALF)
        a1_group(1)

        # fills during X1a flight: s2b, eps term, BN constants. s2b must
        # precede a2's psum allocation (acc-tag rotation order).
        if EB < NB:
            s2_blocks(EB, NB, "b")
        ehxT = med.tile([D, HALF], bf16, tag="ehxT")
        nc.vector.tensor_scalar_mul(ehxT[:], hxwT[:], eps_t[:])
        s_bn = const.tile([D, 1], fp32, tag="s_bn")
        nc.vector.tensor_scalar_add(s_bn[:], bnv_t[:], BN_EPS)
        nc.scalar.activation(s_bn[:], s_bn[:], Act.Sqrt)
        nc.vector.reciprocal(s_bn[:], s_bn[:])
        nc.vector.tensor_mul(s_bn[:], s_bn[:], bng_t[:])
        t_bn = const.tile([D, 1], fp32, tag="t_bn")
        nc.vector.tensor_mul(t_bn[:], bnm_t[:], s_bn[:])
        nc.vector.tensor_tensor(t_bn[:], bnb_t[:], t_bn[:], op=Alu.subtract)

        # stream chunks: 2MB (4 tiles bf16), alternate sync/scalar queues
        def stream_chunk(dten, c, nm):
            t = st.tile([128, 4 * HALF], bf16, tag="st", name=f"{nm}{c}")
            nc.sync.dma_start(
                out=t[:], in_=dten.ap()[:, c * 4 * HALF:(c + 1) * 4 * HALF])
            return t

        # ------- a2: h1bT[d, n_own] = (Dv @ h1a).T, contraction full n ----
        # Remote-first: X1a (group 0) yields the LF-second-half (n_other)
        # stationaries, consumed while X1b is still in flight.
        h1an = nt1.tile([128, N], bf16, tag="nt1", name="h1an")
        h1b_ps = [acc_t(b, f"h1b{b}") for b in range(NB)]
        dv_chunks = {}

        def half_pass(ps_list, statn, chunks, dten, nm, lo, first, last):
            for j in range(HT):
                jj = lo + j
                c = jj // 4
                if c not in chunks:
                    chunks[c] = stream_chunk(dten, c, nm)
                mv = chunks[c][:, (jj % 4) * HALF:(jj % 4 + 1) * HALF]
                for b in range(NB):
                    nc.tensor.matmul(ps_list[b][:],
                                     statn[:, jj * 128:(jj + 1) * 128],
                                     mv[:, b * F:(b + 1) * F],
                                     start=(first and j == 0),
                                     stop=(last and j == HT - 1))

        def x1_combine(g):
            """Receive X1 group g: combined = sum_g - own_g + own_otherhalf;
            transpose into the OTHER LF half's stationaries. Two pipelined
            half-width pieces so transposes start as soon as data lands."""
            cout, own_ap, s, r, W = x1[g]
            lo, olo = g * HALF, (1 - g) * HALF
            base = HT * (1 - g)
            for p in range(2):
                c0, c1 = p * HALF // 2, (p + 1) * HALF // 2
                nc.gpsimd.dma_start(out=s[:, c0:c1], in_=cout[:][:, c0:c1])
                nc.vector.tensor_tensor(s[:, c0:c1], s[:, c0:c1],
                                        own_ap[:, c0:c1], op=Alu.subtract)
                nc.vector.tensor_tensor(s[:, c0:c1], s[:, c0:c1],
                                        h1aTf[:, olo + c0:olo + c1],
                                        op=Alu.add)
                for j in range(p * HT // 2, (p + 1) * HT // 2):
                    transpose_to(
                        h1an[:, (base + j) * 128:(base + j + 1) * 128],
                        s[:, j * 128:(j + 1) * 128])

        dv_chunks[4] = stream_chunk(dvt_d, 4, "dv")
        dv_chunks[5] = stream_chunk(dvt_d, 5, "dv")
        x1_combine(0)
        # x1b emitted only now: on the gpsimd queue it must sit AFTER x1a's
        # receive dma (collectives hold the queue until they complete).
        x1[1] = exchange_start("x1b", h1aTf[:, HALF:], HALF)
        half_pass(h1b_ps, h1an, dv_chunks, dvt_d, "dv", HT, True, False)
        x1_combine(1)
        half_pass(h1b_ps, h1an, dv_chunks, dvt_d, "dv", 0, False, True)
        h1bT = tT("h1bT")
        for b in range(NB):
            nc.vector.tensor_copy(h1bT[:, b * F:(b + 1) * F], h1b_ps[b][:])
        e2 = exchange_start("e2", h1bT[:], HALF)

        # ------- a3: h1cT[d, e_own] = (Ht @ h1b).T, hcol resident ---------
        h1bn = nt2.tile([128, N], bf16, tag="nt2", name="h1bn")
        for j in range(HT):
            transpose_to(h1bn[:, j * 128:(j + 1) * 128],
                         h1bT[:, j * 128:(j + 1) * 128])
        h1c_ps = [acc_t(b, f"h1c{b}") for b in range(NB)]
        for j in range(HT):
            for b in range(NB):
                nc.tensor.matmul(h1c_ps[b][:],
                                 h1bn[:, j * 128:(j + 1) * 128],
                                 hcol_bap(j, b),
                                 start=(j == 0), stop=False)
        det_chunks = {c: stream_chunk(det_d, c, "de") for c in range(2)}
        recv_transpose(e2, h1bn, HT)
        for j in range(HT):
            jj = HT + j
            for b in range(NB):
                nc.tensor.matmul(h1c_ps[b][:],
                                 h1bn[:, jj * 128:(jj + 1) * 128],
                                 hcol_bap(jj, b),
                                 start=False, stop=(j == HT - 1))
        h1cT = tT("h1cT")
        for b in range(NB):
            nc.vector.tensor_copy(h1cT[:, b * F:(b + 1) * F], h1c_ps[b][:])
        e3 = exchange_start("e3", h1cT[:], HALF)

        # htr5 chunks (a5 tail: own-masked, remote packed, remote masked)
        # load during a4, recycling hc pool bufs (dead after a3); on gpsimd
        # between e3 and e4 (deps are all pre-e3).
        htr5_tiles = []
        if PAD5 > 0:
            for c in range(PAD5 // CH5):
                t = hcA.tile([128, CH5 * HALF], f8e4, tag="hcA",
                             name=f"h5{c}", padded_shape=[128, CH * 3 * F])
                nc.gpsimd.dma_start(
                    out=t[:],
                    in_=htr5_d.ap()[:, c * CH5 * HALF:(c + 1) * CH5 * HALF])
                htr5_tiles.append(t)

        def htr5_ap(t_):
            return htr5_tiles[t_ // CH5][:, (t_ % CH5) * HALF:(t_ % CH5 + 1) * HALF]

        # ------- a4: h1dT[d, e_own] = (De @ h1c).T, contraction full e ----
        h1cn = nt1.tile([128, N], bf16, tag="nt1", name="h1cn")
        for j in range(HT):
            transpose_to(h1cn[:, j * 128:(j + 1) * 128],
                         h1cT[:, j * 128:(j + 1) * 128])
        h1d_ps = [acc_t(b, f"h1d{b}") for b in range(NB)]
        half_pass(h1d_ps, h1cn, det_chunks, det_d, "de", 0, True, False)
        recv_transpose(e3, h1cn, HT)
        half_pass(h1d_ps, h1cn, det_chunks, det_d, "de", HT, False, True)
        # h = h1d + eps * hxw (own e-half)
        hT = tT("hT")
        for b in range(NB):
            nc.vector.tensor_tensor(hT[:, b * F:(b + 1) * F], h1d_ps[b][:],
                                    ehxT[:, b * F:(b + 1) * F], op=Alu.add)
        e4 = exchange_start("e4", hT[:], HALF)

        # ------- a5: outT[d, n_own] = (H @ h).T, contraction full e -------
        hn = nt2.tile([128, N], bf16, tag="nt2", name="hn")
        for j in range(HT):
            transpose_to(hn[:, j * 128:(j + 1) * 128],
                         hT[:, j * 128:(j + 1) * 128])
        out_ps = [acc_t(b, f"out{b}") for b in range(NB)]

        def a5_mm(hidx, mv, first, last):
            for b in range(NB):
                nc.tensor.matmul(out_ps[b][:],
                                 hn[:, hidx * 128:(hidx + 1) * 128],
                                 mv[:, b * F:(b + 1) * F],
                                 start=first, stop=last)

        # own packed: htr1 resident, n_own = LF cols 0:HALF
        for t in range(T1):
            a5_mm(t, htr1_ap(t, 0, HALF), t == 0, False)
        # own masked
        for s in range(T2):
            a5_mm(T1 + s, htr5_ap(s), False, False)
        recv_transpose(e4, hn, HT)
        # remote packed + remote masked
        for t in range(T1):
            a5_mm(HT + t, htr5_ap(T2 + t), False,
                  T2 == 0 and t == T1 - 1)
        for s in range(T2):
            a5_mm(HT + T1 + s, htr5_ap(T2 + T1 + s), False,
                  s == T2 - 1)

        # ------- epilogue: y = bn(leaky_relu(out)), per psum block --------
        outT = tT("outT")
        for b in range(NB):
            sl = slice(b * F, (b + 1) * F)
            nc.scalar.activation(outT[:, sl], out_ps[b][:], Act.Lrelu,
                                 alpha=0.01)
            nc.vector.tensor_scalar(outT[:, sl], outT[:, sl], s_bn[:],
                                    t_bn[:], op0=Alu.mult, op1=Alu.add)
            nc.sync.dma_start(out=y_d.ap()[:, sl], in_=outT[:, sl])

    nc.finalize()
    return nc


def _tiled(a, ntiles, width):
    """[ntiles*128, width] -> [128, ntiles*width] tiled-major layout."""
    return np.ascontiguousarray(
        a.reshape(ntiles, 128, width).transpose(1, 0, 2)
        .reshape(128, ntiles * width))


def _prepare(inputs):
    from ml_dtypes import bfloat16, float8_e4m3

    H = np.asarray(inputs["incident_mat"], dtype=np.float32)
    Dv = np.asarray(inputs["degree_v"], dtype=np.float32)
    De = np.asarray(inputs["degree_e"], dtype=np.float32)
    x = np.asarray(inputs["x"], dtype=np.float32)
    em = np.asarray(inputs["e_masks"])
    w = np.ascontiguousarray(
        np.asarray(inputs["mlp_W"], dtype=np.float32).astype(bfloat16))
    th = np.asarray(inputs["theta_att"], dtype=np.float32).reshape(D, 1)
    eps = np.full((D, 1), float(np.asarray(inputs["eps"]).reshape(-1)[0]),
                  dtype=np.float32)

    def col(v):
        return np.ascontiguousarray(
            np.asarray(v, dtype=np.float32).reshape(D, 1))

    bng, bnb = col(inputs["bn_gamma"]), col(inputs["bn_beta"])
    bnm, bnv = col(inputs["bn_mean"]), col(inputs["bn_var"])

    perms = []
    maxcnt = 0
    for g in range(B):
        unm = em[g] != 0
        ph = []
        for h in range(2):
            idx = np.arange(h * HALF, (h + 1) * HALF)
            m = unm[idx]
            ph.append(np.concatenate([idx[m], idx[~m]]))
            maxcnt = max(maxcnt, int(m.sum()))
        perms.append(ph)
    T1 = min(HT, (maxcnt + 127) // 128)
    T2 = HT - T1
    A5T = 2 * T2 + T1
    CH5 = 6                 # htr5 tiles per chunk (1.5MB, recycles hcA bufs)
    PAD5 = ((A5T + CH5 - 1) // CH5) * CH5
    H1C = (T1 + 1) // 2

    key = ("nc", T1)
    if key not in _CACHE:
        _CACHE[key] = _build(T1)
    nc = _CACHE[key]

    in_maps = []
    for g in range(B):
        Hg = H[g]
        HgT = np.ascontiguousarray(Hg.T)
        xg = x[g]
        xth = (xg.astype(np.float64) @ th.astype(np.float64)).reshape(-1)
        scores = Hg.astype(np.float64).T @ xth
        scores = np.where(em[g] == 0, -np.inf, scores)
        scores -= scores.max()
        ex = np.exp(scores)
        attn = (ex / ex.sum()).astype(np.float32)
        for h in range(2):
            po, pr = perms[g][h], perms[g][1 - h]
            n_own = slice(h * HALF, (h + 1) * HALF)
            n_lf = np.concatenate([np.arange(h * HALF, (h + 1) * HALF),
                                   np.arange((1 - h) * HALF, (2 - h) * HALF)])
            e_lf = np.concatenate([po, pr])
            hfull = Hg[n_lf][:, po].astype(float8_e4m3)
            hfull = hfull.reshape(N, 1, HALF)  # [n, 1, e] for col split
            ap = attn[po[:T1 * 128]].reshape(T1, 128).T
            # htr1: own packed e rows x full n (LF cols), padded to chunk
            h1 = np.zeros((2 * H1C * 128, N), dtype=float8_e4m3)
            h1[:T1 * 128] = HgT[po[:T1 * 128]][:, n_lf].astype(float8_e4m3)
            # htr5: [own masked | remote packed | remote masked] x n_own
            e5 = np.concatenate([po[T1 * 128:], pr[:T1 * 128],
                                 pr[T1 * 128:]])
            h5 = np.zeros((max(1, PAD5) * 128, HALF), dtype=float8_e4m3)
            h5[:A5T * 128] = HgT[e5, n_own].astype(float8_e4m3)
            m = {
                "xt": _tiled(xg[n_lf].astype(bfloat16), NT, 128),
                "attn": np.ascontiguousarray(ap),
                "hcolA": _tiled(hfull[:, :, :3 * F].reshape(N, 3 * F),
                                NT, 3 * F),
                "hcolB": _tiled(hfull[:, :, 3 * F:].reshape(N, F), NT, F),
                "htr1": _tiled(h1, 2 * H1C, N),
                "htr5": _tiled(h5, max(1, PAD5), HALF),
                "dvt": _tiled(Dv[g].T[n_lf][:, n_own].astype(bfloat16),
                              NT, HALF),
                "det": _tiled(De[g].T[e_lf][:, po].astype(bfloat16),
                              NT, HALF),
                "w": w, "eps": eps,
                "bng": bng, "bnb": bnb, "bnm": bnm, "bnv": bnv,
            }
            in_maps.append(m)
    return nc, in_maps


def kernel(**inputs):
    from concourse.bass_utils import run_bass_kernel_spmd

    nc, in_maps = _prepare(inputs)
    res = run_bass_kernel_spmd(nc, in_maps, list(range(NCORES)))
    out = np.empty((B, N, D), dtype=np.float32)
    for g in range(B):
        for h in range(2):
            ya = res.results[2 * g + h]["y"].astype(np.float32)
            out[g, h * HALF:(h + 1) * HALF, :] = ya.T
    return out
